# revision 1
# baseline (speedup 1.0000x reference)
"""DFEM kernel for 8 TRN2 NeuronCores.

Data-parallel over batch B=8: core b computes sample b end-to-end
(conv1x1 -> spatial-attention weight, PAM self-attention on both inputs,
final combine). No collectives.

Shapes (hardcoded): B=8, C=256, C8=32, H=W=64, N=4096.

Attention is computed transposed: energy^T chunks [j,i] = k_chunk^T @ q,
exp on ScalarE (logits are tiny, no max subtraction needed), softmax
denominator Z[i] via ones-vector matmul on TensorE, PV via v^T (computed
directly in transposed layout), normalization folded into the epilogue.
"""

import numpy as np
import ml_dtypes

BF16 = ml_dtypes.bfloat16

B, C, C8, H, W = 8, 256, 32, 64, 64
N = H * W          # 4096
P = 128            # partitions
NCT = C // P       # 2 c-tiles
NB = 512           # i-block size
NIB = N // NB      # 8 i-blocks
JB = 128           # j-chunk size
NJT = N // JB      # 32 j-chunks

_CACHE = {}


def _build_program():
    import concourse.bacc as bacc
    import concourse.mybir as mybir
    import concourse.tile as tile

    f32 = mybir.dt.float32
    bf16 = mybir.dt.bfloat16
    fp8 = mybir.dt.float8e4
    DR = mybir.MatmulPerfMode.DoubleRow
    AF = mybir.ActivationFunctionType
    ALU = mybir.AluOpType

    nc = bacc.Bacc("TRN2", target_bir_lowering=False, debug=False, num_devices=B)

    # ---- DRAM I/O ----
    x1f = nc.dram_tensor("x1f", (C, N), f32, kind="ExternalInput")
    x1b = nc.dram_tensor("x1b", (C, N), bf16, kind="ExternalInput")
    x2f = nc.dram_tensor("x2f", (C, N), f32, kind="ExternalInput")
    x2b = nc.dram_tensor("x2b", (C, N), bf16, kind="ExternalInput")
    w1T = nc.dram_tensor("w1T", (C, C), bf16, kind="ExternalInput")
    wqT = nc.dram_tensor("wqT", (C, C8), bf16, kind="ExternalInput")
    wkT = nc.dram_tensor("wkT", (C, C8), bf16, kind="ExternalInput")
    wvT = nc.dram_tensor("wvT", (C, C), bf16, kind="ExternalInput")
    b1c = nc.dram_tensor("b1c", (C, 1), f32, kind="ExternalInput")
    bqc = nc.dram_tensor("bqc", (C8, 1), f32, kind="ExternalInput")
    bkc = nc.dram_tensor("bkc", (C8, 1), f32, kind="ExternalInput")
    bv_rep = nc.dram_tensor("bv_rep", (P, C), f32, kind="ExternalInput")
    gamma_s = nc.dram_tensor("gamma_s", (P, 1), f32, kind="ExternalInput")
    wsa_rep = nc.dram_tensor("wsa_rep", (64, 18), f32, kind="ExternalInput")
    ones_c = nc.dram_tensor("ones_c", (P, 2 * P), fp8, kind="ExternalInput")
    ones_b = nc.dram_tensor("ones_b", (P, 1), bf16, kind="ExternalInput")
    out_d = nc.dram_tensor("out", (C, N), f32, kind="ExternalOutput")


    def ct_tiles(ap):  # [C, N] -> [2, 128, N]
        return ap.rearrange("(t p) n -> t p n", p=P)

    x1f_t, x1b_t = ct_tiles(x1f), ct_tiles(x1b)
    x2f_t, x2b_t = ct_tiles(x2f), ct_tiles(x2b)
    w1T_t, wvT_t = ct_tiles(w1T), ct_tiles(wvT)
    wqT_t, wkT_t = ct_tiles(wqT), ct_tiles(wkT)
    b1c_t = b1c.rearrange("(t p) o -> t p o", p=P)
    out_dt = ct_tiles(out_d)

    with tile.TileContext(nc) as tc:
        from contextlib import ExitStack
        with ExitStack() as ctx:
            consts = ctx.enter_context(tc.tile_pool(name="consts", bufs=1))
            persist = ctx.enter_context(tc.tile_pool(name="persist", bufs=1))
            stream = ctx.enter_context(tc.tile_pool(name="stream", bufs=2))
            cstream = ctx.enter_context(tc.tile_pool(name="cstream", bufs=6))
            apool = ctx.enter_context(tc.tile_pool(name="apool", bufs=6))
            ps512 = ctx.enter_context(tc.tile_pool(name="ps512", bufs=2, space="PSUM"))
            pvps = ctx.enter_context(tc.tile_pool(name="pvps", bufs=3, space="PSUM"))
            zps = ctx.enter_context(tc.tile_pool(name="zps", bufs=1, space="PSUM"))

            # ---- load constants ----
            def cload(ap, shape, dtype, tag):
                t = consts.tile(shape, dtype, tag=tag, name=tag)
                nc.sync.dma_start(out=t, in_=ap)
                return t

            w1T_s = [cload(w1T_t[i], [P, C], bf16, f"w1T{i}") for i in range(NCT)]
            wqT_s = [cload(wqT_t[i], [P, C8], bf16, f"wqT{i}") for i in range(NCT)]
            wkT_s = [cload(wkT_t[i], [P, C8], bf16, f"wkT{i}") for i in range(NCT)]
            wvT_s = [cload(wvT_t[i], [P, C], bf16, f"wvT{i}") for i in range(NCT)]
            b1_s = [cload(b1c_t[i], [P, 1], f32, f"b1{i}") for i in range(NCT)]
            bq_s = cload(bqc[:, :], [C8, 1], f32, "bq")
            bk_s = cload(bkc[:, :], [C8, 1], f32, "bk")
            bv_s = cload(bv_rep[:, :], [P, C], f32, "bv")
            gam_rep = cload(gamma_s[:, :], [P, 1], f32, "gam")
            wsa_s = cload(wsa_rep[:, :], [64, 18], f32, "wsa")
            ones_s = cload(ones_c[:, :], [P, 2 * P], fp8, "ones")
            onesb_s = cload(ones_b[:, :], [P, 1], bf16, "onesb")

            # ---- persistent tiles ----
            x11b = [persist.tile([P, N], bf16, tag=f"x11b{i}", name=f"x11b{i}") for i in range(NCT)]
            x21b = [persist.tile([P, N], bf16, tag=f"x21b{i}", name=f"x21b{i}") for i in range(NCT)]
            q_sb = persist.tile([4 * C8, N], bf16, tag="q_sb", name="q_sb")
            k_sb = persist.tile([4 * C8, N], bf16, tag="k_sb", name="k_sb")
            vT_sb = persist.tile([P, NJT * C], fp8, tag="vT_sb", name="vT_sb")
            out1 = [persist.tile([P, N], f32, tag=f"out1_{i}", name=f"out1_{i}") for i in range(NCT)]
            out2 = [persist.tile([P, N], f32, tag=f"out2_{i}", name=f"out2_{i}") for i in range(NCT)]
            zg_rep = persist.tile([P, N], f32, tag="zg_rep", name="zg_rep")
            # 3 dy-shifted padded planes per channel: plane[ky][h, 1+w] holds
            # image row h+ky-1 (zeros outside). Taps then always read
            # partition base 0 (DVE requires 32-aligned partition offsets).
            planes = [[persist.tile([64, 66], f32, tag=f"plane{c}{k}",
                                    name=f"plane{c}{k}")
                       for k in range(3)] for c in range(2)]
            acc_sa = persist.tile([64, 64], f32, tag="acc_sa", name="acc_sa")
            w64 = persist.tile([64, 64], f32, tag="w64", name="w64")

            # ================= conv1x1 (shared weights) =================
            def conv(xb_dram_t, xout_b):
                # load bf16 input tiles, chunked so matmuls start immediately
                xin = []
                for i in range(NCT):
                    t = stream.tile([P, N], bf16, tag="stream", name="stream")
                    xin.append(t)
                for nb in range(NIB):
                    for i in range(NCT):
                        sl = slice(nb * NB, (nb + 1) * NB)
                        nc.sync.dma_start(out=xin[i][:, sl], in_=xb_dram_t[i][:, sl])
                for nb in range(NIB):
                    for ot in range(NCT):
                        ps = ps512.tile([P, NB], f32, tag="ps512", name="ps512")
                        sl = slice(nb * NB, (nb + 1) * NB)
                        nc.tensor.matmul(ps, w1T_s[0][:, ot * P:(ot + 1) * P],
                                         xin[0][:, sl], start=True, stop=False)
                        nc.tensor.matmul(ps, w1T_s[1][:, ot * P:(ot + 1) * P],
                                         xin[1][:, sl], start=False, stop=True)
                        # biased bf16 copy for downstream matmuls / SA / residual
                        nc.scalar.activation(xout_b[ot][:, sl], ps, AF.Identity,
                                             bias=b1_s[ot][:, 0:1])

            # conv1 stores f32 via out1 tiles, conv2 via out2 tiles
            conv(x1b_t, x11b)
            conv(x2b_t, x21b)

            # ================= PAM attention (one input path) ============
            def qkv(xb):
                for nb in range(NIB):
                    sl = slice(nb * NB, (nb + 1) * NB)
                    for di, (dst, wT, bias) in enumerate(
                            ((q_sb, wqT_s, bq_s), (k_sb, wkT_s, bk_s))):
                        ps = ps512.tile([C8, NB], f32, tag="ps512", name="ps512")
                        nc.tensor.matmul(ps, wT[0], xb[0][:, sl], start=True, stop=False)
                        nc.tensor.matmul(ps, wT[1], xb[1][:, sl], start=False, stop=True)
                        if (nb + di) % 2 == 0:
                            nc.scalar.activation(dst[0:C8, sl], ps, AF.Identity,
                                                 bias=bias[:, 0:1])
                        else:
                            nc.vector.tensor_scalar(dst[0:C8, sl], ps, bias[:, 0:1],
                                                    None, op0=ALU.add)
                for dst in (q_sb, k_sb):
                    nc.sync.dma_start(out=dst[C8:2 * C8, :], in_=dst[0:C8, :])
                    nc.sync.dma_start(out=dst[2 * C8:4 * C8, :], in_=dst[0:2 * C8, :])
                for jt in range(NJT):
                    jsl = slice(jt * JB, (jt + 1) * JB)
                    ps = pvps.tile([P, NB], f32, tag="pvps", name="pvps")
                    nc.tensor.matmul(ps[:, 0:C], xb[0][:, jsl], wvT_s[0],
                                     start=True, stop=False)
                    nc.tensor.matmul(ps[:, 0:C], xb[1][:, jsl], wvT_s[1],
                                     start=False, stop=True)
                    nc.vector.tensor_tensor(
                        vT_sb[:, jt * C:(jt + 1) * C], ps[:, 0:C], bv_s, op=ALU.add)

            def attention(outp, post_ib):
                """energy^T/exp/Z/PV pipeline. Per i-block: unnormalized PV
                into outp tiles, 1/Z (all partitions) into zg_rep. The
                post_ib thunks (residual / combine) are spread one-per-pair
                through the NEXT block's pipeline so no engine queue gets a
                bursty serial chain."""
                NPAIR = NJT // 2
                pending = []
                for ib in range(NIB):
                    isl = slice(ib * NB, (ib + 1) * NB)
                    pv = [pvps.tile([P, NB], f32, tag="pvps", name="pvps") for _ in range(NCT)]
                    zp = zps.tile([1, NB], f32, tag="zps", name="zps")
                    etiles = {}

                    def consume(g):
                        at = apool.tile([P, 2 * NB], fp8, tag="apool", name="apool")
                        ep_t = etiles.pop(g)
                        nc.scalar.activation(at[:, 0:NB], ep_t[:, 0:NB], AF.Exp)
                        nc.vector.tensor_scalar(
                            at.bitcast(mybir.dt.uint8)[:, NB:2 * NB],
                            ep_t[:, NB:2 * NB],
                            11.7724, 55.0, op0=ALU.mult, op1=ALU.add)
                        # DoubleRow: contract both j-chunks of the pair at once
                        atr = at.rearrange("p (r n) -> p r n", r=2)
                        st, sp = (g == 0), (g == NPAIR - 1)
                        for h in range(NCT):
                            vsl = vT_sb[:, 2 * g * C: (2 * g + 2) * C].rearrange(
                                "p (r c) -> p r c", r=2)[:, :, h * P:(h + 1) * P]
                            nc.tensor.matmul(pv[h], vsl, atr, start=st, stop=sp,
                                             perf_mode=DR, skip_group_check=True)
                        onr = ones_s.rearrange("p (r m) -> p r m", r=2)[:, :, 0:1]
                        nc.tensor.matmul(zp, onr, atr, start=st, stop=sp,
                                         perf_mode=DR, skip_group_check=True)

                    for g in range(NPAIR):
                        # two j-chunks concurrently on two 32-row PE bands
                        ep = ps512.tile([P, 2 * NB], f32, tag="ps512", name="ps512")
                        for half in range(2):
                            jt = 2 * g + half
                            band = slice(half * C8, (half + 1) * C8)
                            nc.tensor.matmul(ep[:, half * NB:(half + 1) * NB],
                                             k_sb[band, jt * JB:(jt + 1) * JB],
                                             q_sb[band, isl], start=True, stop=True,
                                             skip_group_check=True)
                        etiles[g] = ep
                        if g >= 2:
                            consume(g - 2)
                        if pending:
                            pending.pop(0)()
                    consume(NPAIR - 2)
                    consume(NPAIR - 1)

                    # reciprocal via [128,4] reshape (all lanes), then bcast chunk
                    zc = cstream.tile([P, NB], f32, tag="cstream", name="cstream")
                    nc.vector.tensor_copy(zg_rep[0:1, isl], zp[0:1, :])
                    nc.sync.dma_start(out=zc[0:P, 0:NB // P], in_=zg_rep[0:1, isl])
                    nc.vector.reciprocal(zc[0:P, 0:NB // P], zc[0:P, 0:NB // P])
                    nc.sync.dma_start(out=zg_rep[0:1, isl], in_=zc[0:P, 0:NB // P])
                    nc.gpsimd.partition_broadcast(zg_rep[:, isl], zg_rep[0:1, isl])
                    for h in range(NCT):
                        nc.vector.tensor_copy(outp[h][:, isl], pv[h])
                    pending = post_ib(ib, isl)
                for th in pending:
                    th()

            def residual_thunks(outp, xb_res, isl):
                # outp = (pam*gamma)/Z + x_conv (biased bf16, resident);
                # scale reads the PV accumulator straight from PSUM
                def scale(t):
                    return lambda: nc.vector.scalar_tensor_tensor(
                        outp[t][:, isl], outp[t][:, isl], gam_rep[:, 0:1],
                        zg_rep[:, isl], op0=ALU.mult, op1=ALU.mult)
                def add(t):
                    return lambda: nc.vector.tensor_tensor(
                        outp[t][:, isl], outp[t][:, isl], xb_res[t][:, isl],
                        op=ALU.add)
                return [scale(0), add(0), scale(1), add(1)]

            def combine(ib, isl):
                ths = residual_thunks(out2, x21b, isl)
                # spatial-attention weight chunk, broadcast to 128 partitions
                wb = cstream.tile([P, NB], f32, tag="cstream", name="cstream")
                nc.sync.dma_start(out=wb[0:1, 0:NB], in_=w64[ib * 8:(ib + 1) * 8, 0:64])
                nc.gpsimd.partition_broadcast(wb, wb[0:1, :])
                for t in range(NCT):
                    a = cstream.tile([P, NB], f32, tag="cstream", name="cstream")
                    b = cstream.tile([P, NB], f32, tag="cstream", name="cstream")
                    nc.sync.dma_start(out=a, in_=x1f_t[t][:, isl])
                    nc.sync.dma_start(out=b, in_=x2f_t[t][:, isl])
                    o1, o2 = out1[t][:, isl], out2[t][:, isl]
                    def block(t=t, a=a, b=b, o1=o1, o2=o2):
                        nc.vector.tensor_tensor(o1, o1, a, op=ALU.mult)
                        nc.vector.tensor_tensor(o2, o2, b, op=ALU.mult)
                    def block2(t=t, o1=o1, o2=o2, wb=wb):
                        nc.vector.tensor_tensor(o1, o2, o1, op=ALU.subtract)
                        # |d| = max(d, -d)
                        nc.vector.scalar_tensor_tensor(o1, o1, -1.0, o1,
                                                       op0=ALU.mult, op1=ALU.max)
                    def block3(t=t, o1=o1, wb=wb, sl2=isl):
                        nc.vector.tensor_tensor(o1, o1, wb, op=ALU.mult)
                        nc.sync.dma_start(out=out_dt[t][:, sl2], in_=o1)
                    ths += [block, block2, block3]
                return ths

            def epilogue(outp, xf_dram_t):
                # reciprocal with all 128 lanes via [128,32] reshape round-trip
                nc.sync.dma_start(out=zcol, in_=zg_rep[0:1, 0:N])
                nc.vector.reciprocal(zcol, zcol)
                nc.sync.dma_start(out=zg_rep[0:1, 0:N], in_=zcol)
                nc.gpsimd.partition_broadcast(zg_rep, zg_rep[0:1, :])
                EB = 2 * NB
                for t in range(NCT):
                    for cb in range(N // EB):
                        sl = slice(cb * EB, (cb + 1) * EB)
                        st = cstream.tile([P, EB], f32, tag="cstream", name="cstream")
                        nc.sync.dma_start(out=st, in_=xf_dram_t[t][:, sl])
                        # outp = (pam_unnorm * gamma) * (1/Z) then + (x11 + b1)
                        nc.vector.scalar_tensor_tensor(
                            outp[t][:, sl], outp[t][:, sl], gam_rep[:, 0:1],
                            zg_rep[:, sl], op0=ALU.mult, op1=ALU.mult)
                        nc.vector.scalar_tensor_tensor(
                            outp[t][:, sl], st, b1_s[t][:, 0:1], outp[t][:, sl],
                            op0=ALU.add, op1=ALU.add)

            qkv(x11b)
            # ================= spatial attention weight ==================
            # mean over 512 channels via ones-matmul (scaled by 1/512)
            for nb in range(NIB):
                sl = slice(nb * NB, (nb + 1) * NB)
                mp = zps.tile([1, NB], f32, tag="zps", name="zps")
                first = True
                for srcb in (x11b[0], x11b[1], x21b[0], x21b[1]):
                    nc.tensor.matmul(mp, onesb_s, srcb[:, sl],
                                     start=first, stop=(srcb is x21b[1]))
                    first = False
                nc.scalar.activation(out2[0][0:1, sl], mp[0:1, :], AF.Identity,
                                     scale=1.0 / (2 * C))
            # max over 512 channels: pairwise DVE max then partition all-reduce
            nc.vector.tensor_tensor(out2[1], x11b[0], x11b[1], op=ALU.max)
            nc.vector.tensor_tensor(out2[1], out2[1], x21b[0], op=ALU.max)
            nc.vector.tensor_tensor(out2[1], out2[1], x21b[1], op=ALU.max)
            import concourse.bass_isa as bass_isa
            nc.gpsimd.partition_all_reduce(out1[0], out2[1], channels=P,
                                           reduce_op=bass_isa.ReduceOp.max)

            # 3x3 conv (2->1 ch) + sigmoid on the 64x64 grid
            for ci, row in ((0, out2[0]), (1, out1[0])):
                img = row[0:1, 0:N].rearrange("p (h w) -> p h w", h=64)
                for ky in range(3):
                    pl = planes[ci][ky]
                    nc.vector.memset(pl, 0.0)
                    if ky == 0:    # plane rows 1..63 <- image rows 0..62
                        nc.sync.dma_start(out=pl[1:64, 1:65], in_=img[:, 0:63, :])
                    elif ky == 1:  # plane rows 0..63 <- image rows 0..63
                        nc.sync.dma_start(out=pl[0:64, 1:65], in_=img[:, 0:64, :])
                    else:          # plane rows 0..62 <- image rows 1..63
                        nc.sync.dma_start(out=pl[0:63, 1:65], in_=img[:, 1:64, :])
            tap = 0
            for ci in range(2):
                for ky in range(3):
                    for kx in range(3):
                        wcol = wsa_s[0:64, tap:tap + 1]
                        window = planes[ci][ky][0:64, kx:kx + 64]
                        if tap == 0:
                            nc.vector.tensor_scalar_mul(acc_sa, window, wcol)
                        else:
                            nc.vector.scalar_tensor_tensor(
                                acc_sa, window, wcol, acc_sa,
                                op0=ALU.mult, op1=ALU.add)
                        tap += 1
            nc.scalar.activation(w64, acc_sa, AF.Sigmoid)

            attention(out1, lambda ib, isl: residual_thunks(out1, x11b, isl))
            qkv(x21b)
            attention(out2, combine)

    nc.compile()
    return nc


def _prep_inputs(x1, x2, w1, b1, wq, bq, wk, bk, wv, bv, gamma, w_sa):
    shared = {
        "w1T": np.ascontiguousarray(w1.T).astype(BF16),
        "wqT": np.ascontiguousarray(wq.T).astype(BF16),
        "wkT": np.ascontiguousarray(wk.T).astype(BF16),
        "wvT": np.ascontiguousarray(wv.T).astype(BF16),
        "b1c": np.ascontiguousarray(b1.reshape(C, 1)).astype(np.float32),
        "bqc": np.ascontiguousarray(bq.reshape(C8, 1)).astype(np.float32),
        "bkc": np.ascontiguousarray(bk.reshape(C8, 1)).astype(np.float32),
        "bv_rep": np.broadcast_to(bv.reshape(1, C), (P, C)).copy().astype(np.float32),
        "gamma_s": np.broadcast_to(np.asarray(gamma, np.float32).reshape(1, 1), (P, 1)).copy(),
        "wsa_rep": np.broadcast_to(
            np.asarray(w_sa, np.float32).reshape(1, 18), (64, 18)).copy(),
        "ones_c": np.ones((P, 2 * P), ml_dtypes.float8_e4m3),
        "ones_b": np.ones((P, 1), BF16),
    }
    in_maps = []
    for bidx in range(B):
        x1s = np.ascontiguousarray(x1[bidx].reshape(C, N)).astype(np.float32)
        x2s = np.ascontiguousarray(x2[bidx].reshape(C, N)).astype(np.float32)
        m = dict(shared)
        m["x1f"] = x1s
        m["x1b"] = x1s.astype(BF16)
        m["x2f"] = x2s
        m["x2b"] = x2s.astype(BF16)
        in_maps.append(m)
    return in_maps


def kernel(x1, x2, w1, b1, wq, bq, wk, bk, wv, bv, gamma, w_sa, _trace=False):
    from concourse.bass_utils import run_bass_kernel_spmd

    if "nc" not in _CACHE:
        _CACHE["nc"] = _build_program()
    nc = _CACHE["nc"]

    in_maps = _prep_inputs(np.asarray(x1), np.asarray(x2), np.asarray(w1),
                           np.asarray(b1), np.asarray(wq), np.asarray(bq),
                           np.asarray(wk), np.asarray(bk), np.asarray(wv),
                           np.asarray(bv), np.asarray(gamma), np.asarray(w_sa))
    res = run_bass_kernel_spmd(nc, in_maps, core_ids=list(range(B)), trace=_trace)
    _CACHE["last_result"] = res
    out = np.stack([res.results[c]["out"] for c in range(B)], axis=0)
    return out.reshape(B, C, H, W).astype(np.float32)



# revision 10
# speedup vs baseline: 1.1448x; 1.1448x over previous
"""DFEM kernel for 8 TRN2 NeuronCores.

Data-parallel over batch B=8: core b computes sample b end-to-end
(conv1x1 -> spatial-attention weight, PAM self-attention on both inputs,
final combine). No collectives.

Shapes (hardcoded): B=8, C=256, C8=32, H=W=64, N=4096.

Attention is computed transposed: energy^T chunks [j,i] = k_chunk^T @ q
with 4-band row rotation (PE row groups alternate between consecutive
pair-groups so LDWEIGHTS pulls ahead), exp whole-pair on ScalarE (table)
or VectorE (fp8 bit-trick), PV via v^T fp8 DoubleRow, softmax Z via
ones-vector matmul, normalization + residual + combine in bf16 (2x DVE
mode). The v-projection bias is folded algebraically into the residual
(sum_j A[j,i] = Z[i], so bv*Z/Z = bv exactly).
"""

import numpy as np
import ml_dtypes

BF16 = ml_dtypes.bfloat16

B, C, C8, H, W = 8, 256, 32, 64, 64
N = H * W          # 4096
P = 128            # partitions
NCT = C // P       # 2 c-tiles
NB = 512           # i-block size
NIB = N // NB      # 8 i-blocks
JB = 128           # j-chunk size
NJT = N // JB      # 32 j-chunks
NPAIR = NJT // 2   # 16 pair-groups per i-block

# exp engine split per block: groups with (g % 16) in SCALAR_GROUPS go to
# ScalarE table-exp, the rest to the DVE bit-trick.
SCALAR_EXP = frozenset((0, 1, 3, 5, 6, 8, 10, 11, 13, 15))

_CACHE = {}


def _build_program():
    import concourse.bacc as bacc
    import concourse.mybir as mybir
    import concourse.tile as tile

    f32 = mybir.dt.float32
    bf16 = mybir.dt.bfloat16
    fp8 = mybir.dt.float8e4
    u8 = mybir.dt.uint8
    DR = mybir.MatmulPerfMode.DoubleRow
    AF = mybir.ActivationFunctionType
    ALU = mybir.AluOpType

    nc = bacc.Bacc("TRN2", target_bir_lowering=False, debug=False, num_devices=B)

    # ---- DRAM I/O ----
    x1b = nc.dram_tensor("x1b", (C, N), bf16, kind="ExternalInput")
    x2b = nc.dram_tensor("x2b", (C, N), bf16, kind="ExternalInput")
    w1T = nc.dram_tensor("w1T", (C, C), bf16, kind="ExternalInput")
    wqT = nc.dram_tensor("wqT", (C, C8), bf16, kind="ExternalInput")
    wkT = nc.dram_tensor("wkT", (C, C8), bf16, kind="ExternalInput")
    wvT = nc.dram_tensor("wvT", (C, C), bf16, kind="ExternalInput")
    b1c = nc.dram_tensor("b1c", (C, 1), f32, kind="ExternalInput")
    bqc = nc.dram_tensor("bqc", (C8, 1), f32, kind="ExternalInput")
    bkc = nc.dram_tensor("bkc", (C8, 1), f32, kind="ExternalInput")
    gbv = nc.dram_tensor("gbv", (C, 1), f32, kind="ExternalInput")   # gamma*bv
    gam_inv = nc.dram_tensor("gam_inv", (P, 1), f32, kind="ExternalInput")
    wsa_rep = nc.dram_tensor("wsa_rep", (64, 18), f32, kind="ExternalInput")
    ones_c = nc.dram_tensor("ones_c", (P, 2 * P), fp8, kind="ExternalInput")
    ones_b = nc.dram_tensor("ones_b", (P, 1), bf16, kind="ExternalInput")
    out_d = nc.dram_tensor("out", (C, N), f32, kind="ExternalOutput")

    def ct_tiles(ap):  # [C, N] -> [2, 128, N]
        return ap.rearrange("(t p) n -> t p n", p=P)

    x1b_t, x2b_t = ct_tiles(x1b), ct_tiles(x2b)
    w1T_t, wvT_t = ct_tiles(w1T), ct_tiles(wvT)
    wqT_t, wkT_t = ct_tiles(wqT), ct_tiles(wkT)
    b1c_t = b1c.rearrange("(t p) o -> t p o", p=P)
    gbv_t = gbv.rearrange("(t p) o -> t p o", p=P)
    out_dt = ct_tiles(out_d)

    with tile.TileContext(nc) as tc:
        from contextlib import ExitStack
        with ExitStack() as ctx:
            consts = ctx.enter_context(tc.tile_pool(name="consts", bufs=1))
            persist = ctx.enter_context(tc.tile_pool(name="persist", bufs=1))
            cstream = ctx.enter_context(tc.tile_pool(name="cstream", bufs=6))
            fstage = ctx.enter_context(tc.tile_pool(name="fstage", bufs=4))
            apool = ctx.enter_context(tc.tile_pool(name="apool", bufs=6))
            ps512 = ctx.enter_context(tc.tile_pool(name="ps512", bufs=2, space="PSUM"))
            pvps = ctx.enter_context(tc.tile_pool(name="pvps", bufs=3, space="PSUM"))
            zps = ctx.enter_context(tc.tile_pool(name="zps", bufs=1, space="PSUM"))

            # ---- persistent inputs (first: conv starts as soon as chunk 0 lands)
            x1s = [persist.tile([P, N], bf16, tag=f"x1s{i}", name=f"x1s{i}") for i in range(NCT)]
            x2s = [persist.tile([P, N], bf16, tag=f"x2s{i}", name=f"x2s{i}") for i in range(NCT)]
            for dst, src in ((x1s, x1b_t), (x2s, x2b_t)):
                for nb in range(NIB):
                    sl = slice(nb * NB, (nb + 1) * NB)
                    for i in range(NCT):
                        nc.sync.dma_start(out=dst[i][:, sl], in_=src[i][:, sl])

            # ---- load constants ----
            def cload(ap, shape, dtype, tag):
                t = consts.tile(shape, dtype, tag=tag, name=tag)
                nc.sync.dma_start(out=t, in_=ap)
                return t

            w1T_s = [cload(w1T_t[i], [P, C], bf16, f"w1T{i}") for i in range(NCT)]
            wqT_s = [cload(wqT_t[i], [P, C8], bf16, f"wqT{i}") for i in range(NCT)]
            wkT_s = [cload(wkT_t[i], [P, C8], bf16, f"wkT{i}") for i in range(NCT)]
            wvT_s = [cload(wvT_t[i], [P, C], bf16, f"wvT{i}") for i in range(NCT)]
            b1_s = [cload(b1c_t[i], [P, 1], f32, f"b1{i}") for i in range(NCT)]
            gbv_s = [cload(gbv_t[i], [P, 1], f32, f"gbv{i}") for i in range(NCT)]
            bq_s = cload(bqc[:, :], [C8, 1], f32, "bq")
            bk_s = cload(bkc[:, :], [C8, 1], f32, "bk")
            gaminv_s = cload(gam_inv[:, :], [P, 1], f32, "gaminv")
            wsa_s = cload(wsa_rep[:, :], [64, 18], f32, "wsa")
            ones_s = cload(ones_c[:, :], [P, 2 * P], fp8, "ones")
            onesb_s = cload(ones_b[:, :], [P, 1], bf16, "onesb")

            # ---- persistent tiles ----
            x11b = [persist.tile([P, N], bf16, tag=f"x11b{i}", name=f"x11b{i}") for i in range(NCT)]
            x21b = [persist.tile([P, N], bf16, tag=f"x21b{i}", name=f"x21b{i}") for i in range(NCT)]
            q1_sb = persist.tile([4 * C8, N], bf16, tag="q1_sb", name="q1_sb")
            k1_sb = persist.tile([4 * C8, N], bf16, tag="k1_sb", name="k1_sb")
            q2_sb = persist.tile([4 * C8, N], bf16, tag="q2_sb", name="q2_sb")
            k2_sb = persist.tile([4 * C8, N], bf16, tag="k2_sb", name="k2_sb")
            vT1_sb = persist.tile([P, NJT * C], fp8, tag="vT1_sb", name="vT1_sb")
            vT2_sb = persist.tile([P, NJT * C], fp8, tag="vT2_sb", name="vT2_sb")
            out1 = [persist.tile([P, N], bf16, tag=f"out1_{i}", name=f"out1_{i}") for i in range(NCT)]
            out2 = [persist.tile([P, N], bf16, tag=f"out2_{i}", name=f"out2_{i}") for i in range(NCT)]
            zgb = persist.tile([P, N], bf16, tag="zgb", name="zgb")
            # 3 dy-shifted padded planes per channel (bf16), see baseline
            planes = [[persist.tile([64, 66], bf16, tag=f"plane{c}{k}",
                                    name=f"plane{c}{k}")
                       for k in range(3)] for c in range(2)]
            acc_sa = persist.tile([64, 64], f32, tag="acc_sa", name="acc_sa")
            w64 = persist.tile([64, 64], bf16, tag="w64", name="w64")
            # mrow: SA mean row in phase 1, then Z/gamma row during attention
            mrow = persist.tile([1, N], bf16, tag="mrow", name="mrow")
            maxr = persist.tile([P, N], bf16, tag="maxr", name="maxr")

            # ================= conv1x1 (shared weights) =================
            def conv(xin, xout_b):
                for nb in range(NIB):
                    for ot in range(NCT):
                        ps = ps512.tile([P, NB], f32, tag="ps512", name="ps512")
                        sl = slice(nb * NB, (nb + 1) * NB)
                        nc.tensor.matmul(ps, w1T_s[0][:, ot * P:(ot + 1) * P],
                                         xin[0][:, sl], start=True, stop=False)
                        nc.tensor.matmul(ps, w1T_s[1][:, ot * P:(ot + 1) * P],
                                         xin[1][:, sl], start=False, stop=True)
                        # biased bf16 copy for downstream matmuls / SA / residual
                        nc.scalar.activation(xout_b[ot][:, sl], ps, AF.Identity,
                                             bias=b1_s[ot][:, 0:1])

            # ================= PAM projections (one input path) ==========
            def qkv(xb, q_sb, k_sb, vT_sb):
                for nb in range(NIB):
                    sl = slice(nb * NB, (nb + 1) * NB)
                    for di, (dst, wT, bias) in enumerate(
                            ((q_sb, wqT_s, bq_s), (k_sb, wkT_s, bk_s))):
                        ps = ps512.tile([C8, NB], f32, tag="ps512", name="ps512")
                        nc.tensor.matmul(ps, wT[0], xb[0][:, sl], start=True, stop=False)
                        nc.tensor.matmul(ps, wT[1], xb[1][:, sl], start=False, stop=True)
                        if (nb + di) % 2 == 0:
                            nc.scalar.activation(dst[0:C8, sl], ps, AF.Identity,
                                                 bias=bias[:, 0:1])
                        else:
                            nc.vector.tensor_scalar(dst[0:C8, sl], ps, bias[:, 0:1],
                                                    None, op0=ALU.add)
                for dst in (q_sb, k_sb):
                    nc.sync.dma_start(out=dst[C8:2 * C8, :], in_=dst[0:C8, :])
                    nc.sync.dma_start(out=dst[2 * C8:4 * C8, :], in_=dst[0:2 * C8, :])
                for jt in range(NJT):
                    jsl = slice(jt * JB, (jt + 1) * JB)
                    ps = pvps.tile([P, NB], f32, tag="pvps", name="pvps")
                    nc.tensor.matmul(ps[:, 0:C], xb[0][:, jsl], wvT_s[0],
                                     start=True, stop=False)
                    nc.tensor.matmul(ps[:, 0:C], xb[1][:, jsl], wvT_s[1],
                                     start=False, stop=True)
                    # no bias: bv is folded into the residual (bv*Z/Z = bv)
                    if jt % 2 == 0:
                        nc.scalar.activation(vT_sb[:, jt * C:(jt + 1) * C],
                                             ps[:, 0:C], AF.Copy)
                    else:
                        nc.vector.tensor_copy(vT_sb[:, jt * C:(jt + 1) * C],
                                              ps[:, 0:C])

            # ================= spatial attention weight ==================
            def spatial_attention():
                # mean over 512 channels via ones-matmul (scaled by 1/512)
                for nb in range(NIB):
                    sl = slice(nb * NB, (nb + 1) * NB)
                    mp = zps.tile([1, NB], f32, tag="zps", name="zps")
                    first = True
                    for srcb in (x11b[0], x11b[1], x21b[0], x21b[1]):
                        nc.tensor.matmul(mp, onesb_s, srcb[:, sl],
                                         start=first, stop=(srcb is x21b[1]))
                        first = False
                    nc.scalar.activation(mrow[0:1, sl], mp[0:1, :], AF.Identity,
                                         scale=1.0 / (2 * C))
                # max over 512 channels: pairwise DVE max (bf16, 2x mode)
                # then chunked partition all-reduce on gpsimd
                nc.vector.tensor_tensor(maxr, x11b[0], x11b[1], op=ALU.max)
                nc.vector.tensor_tensor(maxr, maxr, x21b[0], op=ALU.max)
                nc.vector.tensor_tensor(maxr, maxr, x21b[1], op=ALU.max)
                import concourse.bass_isa as bass_isa
                for nb in range(NIB):
                    sl = slice(nb * NB, (nb + 1) * NB)
                    nc.gpsimd.partition_all_reduce(
                        out1[0][:, sl], maxr[:, sl], channels=P,
                        reduce_op=bass_isa.ReduceOp.max)

                # 3x3 conv (2->1 ch) + sigmoid on the 64x64 grid
                for ci, row in ((0, mrow), (1, out1[0])):
                    img = row[0:1, 0:N].rearrange("p (h w) -> p h w", h=64)
                    for ky in range(3):
                        pl = planes[ci][ky]
                        nc.vector.memset(pl, 0.0)
                        if ky == 0:    # plane rows 1..63 <- image rows 0..62
                            nc.sync.dma_start(out=pl[1:64, 1:65], in_=img[:, 0:63, :])
                        elif ky == 1:
                            nc.sync.dma_start(out=pl[0:64, 1:65], in_=img[:, 0:64, :])
                        else:          # plane rows 0..62 <- image rows 1..63
                            nc.sync.dma_start(out=pl[0:63, 1:65], in_=img[:, 1:64, :])
                tap = 0
                for ci in range(2):
                    for ky in range(3):
                        for kx in range(3):
                            wcol = wsa_s[0:64, tap:tap + 1]
                            window = planes[ci][ky][0:64, kx:kx + 64]
                            if tap == 0:
                                nc.vector.tensor_scalar_mul(acc_sa, window, wcol)
                            else:
                                nc.vector.scalar_tensor_tensor(
                                    acc_sa, window, wcol, acc_sa,
                                    op0=ALU.mult, op1=ALU.add)
                            tap += 1
                nc.scalar.activation(w64, acc_sa, AF.Sigmoid)

            # ================= attention core ============================
            def attention(outp, q_sb, k_sb, vT_sb, post_ib):
                """energy^T/exp/Z/PV pipeline. Per i-block: unnormalized PV
                copied to bf16 outp tiles via ScalarE, Z -> zrow, 1/Z*gamma
                broadcast to zgb (bf16). post_ib thunks (normalize/residual/
                combine) are spread one-per-group through the NEXT block."""
                pending = []
                for ib in range(NIB):
                    isl = slice(ib * NB, (ib + 1) * NB)
                    pv = [pvps.tile([P, NB], f32, tag="pvps", name="pvps") for _ in range(NCT)]
                    zp = zps.tile([1, NB], f32, tag="zps", name="zps")
                    etiles = {}

                    def consume(g):
                        at = apool.tile([P, 2 * NB], fp8, tag="apool", name="apool")
                        ep_t = etiles.pop(g)
                        if g in SCALAR_EXP:
                            nc.scalar.activation(at, ep_t, AF.Exp)
                        else:
                            nc.vector.tensor_scalar(
                                at.bitcast(u8), ep_t,
                                11.7724, 55.0, op0=ALU.mult, op1=ALU.add)
                        # DoubleRow: contract both j-chunks of the pair at once
                        atr = at.rearrange("p (r n) -> p r n", r=2)
                        st, sp = (g == 0), (g == NPAIR - 1)
                        for h in range(NCT):
                            vsl = vT_sb[:, 2 * g * C: (2 * g + 2) * C].rearrange(
                                "p (r c) -> p r c", r=2)[:, :, h * P:(h + 1) * P]
                            nc.tensor.matmul(pv[h], vsl, atr, start=st, stop=sp,
                                             perf_mode=DR, skip_group_check=True)
                        onr = ones_s.rearrange("p (r m) -> p r m", r=2)[:, :, 0:1]
                        nc.tensor.matmul(zp, onr, atr, start=st, stop=sp,
                                         perf_mode=DR, skip_group_check=True)

                    for g in range(NPAIR):
                        # two j-chunks on two PE row bands; band pair rotates
                        # (0,1)/(2,3) between groups so LDWEIGHTS pulls ahead
                        ep = ps512.tile([P, 2 * NB], f32, tag="ps512", name="ps512")
                        for half in range(2):
                            jt = 2 * g + half
                            band = 2 * (g % 2) + half
                            bsl = slice(band * C8, (band + 1) * C8)
                            nc.tensor.matmul(ep[:, half * NB:(half + 1) * NB],
                                             k_sb[bsl, jt * JB:(jt + 1) * JB],
                                             q_sb[bsl, isl], start=True, stop=True,
                                             tile_position=(band * C8, 0),
                                             skip_group_check=True)
                        etiles[g] = ep
                        if g >= 2:
                            consume(g - 2)
                        if pending:
                            pending.pop(0)()
                    consume(NPAIR - 2)
                    consume(NPAIR - 1)

                    # unnormalized PV -> bf16 out tiles (ScalarE, frees PSUM)
                    for h in range(NCT):
                        nc.scalar.activation(outp[h][:, isl], pv[h], AF.Copy)
                    # Z/gamma -> mrow (bf16); reciprocal via [128,4] reshape
                    # (all lanes); broadcast bf16 chunk to zgb
                    nc.scalar.activation(mrow[0:1, isl], zp[0:1, :], AF.Copy,
                                         scale=gaminv_s[0:1, 0:1])
                    zc = cstream.tile([P, NB // P], bf16, tag="zc", name="zc")
                    zcb = cstream.tile([P, NB // P], bf16, tag="zcb", name="zcb")
                    nc.sync.dma_start(out=zc, in_=mrow[0:1, isl])
                    with nc.allow_low_precision(reason="1/Z in bf16 is ample"):
                        nc.vector.reciprocal(zcb, zc)
                    nc.sync.dma_start(out=zgb[0:1, isl], in_=zcb)
                    nc.gpsimd.partition_broadcast(zgb[:, isl], zgb[0:1, isl])
                    pending = post_ib(ib, isl)
                for th in pending:
                    th()

            def residual_thunks(outp, xres, isl):
                # outp = outp_unnorm * (gamma/Z) + (x_conv + gamma*bv), bf16 2x
                def scale(t):
                    return lambda: nc.vector.tensor_tensor(
                        outp[t][:, isl], outp[t][:, isl], zgb[:, isl], op=ALU.mult)
                def add(t):
                    return lambda: nc.vector.tensor_tensor(
                        outp[t][:, isl], outp[t][:, isl], xres[t][:, isl],
                        op=ALU.add)
                return [scale(0), add(0), scale(1), add(1)]

            def combine(ib, isl):
                # weight chunk broadcast (bf16)
                wb = cstream.tile([P, NB], bf16, tag="wb", name="wb")
                nc.sync.dma_start(out=wb[0:1, 0:NB], in_=w64[ib * 8:(ib + 1) * 8, 0:64])
                nc.gpsimd.partition_broadcast(wb, wb[0:1, :])
                ths = []
                # a_t = out1 * x1 first (out1 is final; independent of zgb)
                for t in range(NCT):
                    o1 = out1[t][:, isl]
                    ths.append(lambda o1=o1, a=x1s[t][:, isl]:
                               nc.vector.tensor_tensor(o1, o1, a, op=ALU.mult))
                ths += residual_thunks(out2, x21b, isl)
                for t in range(NCT):
                    o1, o2 = out1[t][:, isl], out2[t][:, isl]
                    b = x2s[t][:, isl]
                    def bmul(o2=o2, b=b):
                        nc.vector.tensor_tensor(o2, o2, b, op=ALU.mult)
                    def sub(o1=o1, o2=o2):
                        nc.vector.tensor_tensor(o1, o2, o1, op=ALU.subtract)
                    def absf(o1=o1):
                        # |d| = max(d, -d)
                        nc.vector.scalar_tensor_tensor(o1, o1, -1.0, o1,
                                                       op0=ALU.mult, op1=ALU.max)
                    def wmul(t=t, o1=o1, wb=wb, sl2=isl):
                        st = fstage.tile([P, NB], f32, tag="fst", name="fst")
                        nc.vector.tensor_tensor(st, o1, wb, op=ALU.mult)
                        nc.sync.dma_start(out=out_dt[t][:, sl2], in_=st)
                    ths += [bmul, sub, absf, wmul]
                return ths

            # ================= schedule ==================================
            conv(x1s, x11b)
            qkv(x11b, q1_sb, k1_sb, vT1_sb)
            conv(x2s, x21b)
            qkv(x21b, q2_sb, k2_sb, vT2_sb)
            spatial_attention()
            # fold gamma*bv into the residual operands (after SA reads)
            for t in range(NCT):
                nc.vector.tensor_scalar(x11b[t], x11b[t], gbv_s[t][:, 0:1],
                                        None, op0=ALU.add)
                nc.vector.tensor_scalar(x21b[t], x21b[t], gbv_s[t][:, 0:1],
                                        None, op0=ALU.add)

            attention(out1, q1_sb, k1_sb, vT1_sb,
                      lambda ib, isl: residual_thunks(out1, x11b, isl))
            attention(out2, q2_sb, k2_sb, vT2_sb, combine)

    nc.compile()
    return nc


def _prep_inputs(x1, x2, w1, b1, wq, bq, wk, bk, wv, bv, gamma, w_sa):
    gamma_v = float(np.asarray(gamma, np.float32).reshape(-1)[0])
    shared = {
        "w1T": np.ascontiguousarray(w1.T).astype(BF16),
        "wqT": np.ascontiguousarray(wq.T).astype(BF16),
        "wkT": np.ascontiguousarray(wk.T).astype(BF16),
        "wvT": np.ascontiguousarray(wv.T).astype(BF16),
        "b1c": np.ascontiguousarray(b1.reshape(C, 1)).astype(np.float32),
        "bqc": np.ascontiguousarray(bq.reshape(C8, 1)).astype(np.float32),
        "bkc": np.ascontiguousarray(bk.reshape(C8, 1)).astype(np.float32),
        "gbv": (gamma_v * np.asarray(bv, np.float32)).reshape(C, 1).copy(),
        "gam_inv": np.full((P, 1), 1.0 / gamma_v, np.float32),
        "wsa_rep": np.broadcast_to(
            np.asarray(w_sa, np.float32).reshape(1, 18), (64, 18)).copy(),
        "ones_c": np.ones((P, 2 * P), ml_dtypes.float8_e4m3),
        "ones_b": np.ones((P, 1), BF16),
    }
    in_maps = []
    for bidx in range(B):
        m = dict(shared)
        m["x1b"] = np.ascontiguousarray(x1[bidx].reshape(C, N)).astype(BF16)
        m["x2b"] = np.ascontiguousarray(x2[bidx].reshape(C, N)).astype(BF16)
        in_maps.append(m)
    return in_maps


def kernel(x1, x2, w1, b1, wq, bq, wk, bk, wv, bv, gamma, w_sa, _trace=False):
    from concourse.bass_utils import run_bass_kernel_spmd

    if "nc" not in _CACHE:
        _CACHE["nc"] = _build_program()
    nc = _CACHE["nc"]

    in_maps = _prep_inputs(np.asarray(x1), np.asarray(x2), np.asarray(w1),
                           np.asarray(b1), np.asarray(wq), np.asarray(bq),
                           np.asarray(wk), np.asarray(bk), np.asarray(wv),
                           np.asarray(bv), np.asarray(gamma), np.asarray(w_sa))
    res = run_bass_kernel_spmd(nc, in_maps, core_ids=list(range(B)), trace=_trace)
    _CACHE["last_result"] = res
    out = np.stack([res.results[c]["out"] for c in range(B)], axis=0)
    return out.reshape(B, C, H, W).astype(np.float32)
